# revision 1
# baseline (speedup 1.0000x reference)
"""4-bit column-block-quantized linear (ColBlockQuantizedLinear) on 8 TRN2 cores.

Math:  out[b,o] = scales[o] * (sum_i inp[b,i]*wq[o,i] - zeros[o]*rowsum[b])
where wq comes from packed bytes q[o,j] (j = i//2): even i -> low nibble,
odd i -> high nibble.

Device-side identity (all O(O*I) work stays on-device):
    sum_j l*a + sum_j h*b = sum_j q*a + sum_j h*(b-16a)
with q = 16h + l, a[j]=inp[:,2j], b[j]=inp[:,2j+1].

The h-stream never materializes h as an integer: a 4-instruction uint16
bit-trick on DVE writes the bf16 BIT PATTERN 0x4300|(h<<3) (= value 128+8h,
linear in h) at 4x DVE mode.  The matmul pairs it with c' = (b-16a)/8 and the
constant 128*sum(c') falls out as a rank-1 correction row.  The q-stream is a
plain u8->bf16 cast (exact, 0..255) split across ACT and GPSIMD.  Activations
are hi/lo bf16-split so the bf16 matmuls give ~fp32 accuracy; zeros*rowsum and
the 128-offset are a K=6 correction matmul with hi/lo-split factors.

Host byte layout: per core the packed bytes [2048, 1376] are column-paired as
(m, 688+m) into uint16 [2048, 688], so the bit-trick's two output streams land
contiguously in natural column order; the q-cast output is column-interleaved
and its matmuls read it through stride-2 APs.

Sharding: column-parallel over out_features (1376 rows/core), inputs
replicated; per-core output [16,1376] gathered on host.
"""

import numpy as np
import ml_dtypes

B = 16
I = 4096
O = 11008
NCORES = 8
OS = O // NCORES          # 1376 out-features per core
HOS = OS // 2             # 688, u16-packed column count
HALF = I // 2             # 2048 packed columns
KT = HALF // 128          # 16 contraction tiles
# psum-bank o-blocks, each a single arithmetic progression in the interleaved
# q-cast layout (no block crosses the 688-column half boundary)
BLKS = [(0, 512), (512, 176), (688, 512), (1200, 176)]
N_ACT_CAST = 10           # q-cast tiles on ACT; rest on GPSIMD

BF16 = ml_dtypes.bfloat16

_CACHE = {}


def _split_hi_lo(x64):
    """Split float64 array into (hi, lo) bf16 parts: hi+lo ~= x to ~2^-17."""
    hi = x64.astype(BF16)
    lo = (x64 - hi.astype(np.float64)).astype(BF16)
    return hi, lo


def _qcast_ap(qb, s, n):
    """Stride-2 AP over the interleaved q-cast tile covering natural columns
    [s, s+n) (s,n within one half)."""
    if s < HOS:
        return qb[:, 2 * s : 2 * (s + n) : 2]
    return qb[:, 2 * (s - HOS) + 1 : 2 * (s - HOS + n) : 2]


def _build_program():
    import concourse.bacc as bacc
    import concourse.mybir as mybir
    import concourse.tile as tile

    dt = mybir.dt
    op = mybir.AluOpType
    nc = bacc.Bacc("TRN2", target_bir_lowering=False)

    q = nc.dram_tensor("q", [HALF, HOS], dt.uint16, kind="ExternalInput")
    statA = nc.dram_tensor("statA", [128, KT * 64], dt.bfloat16, kind="ExternalInput")
    statC = nc.dram_tensor("statC", [128, KT * 64], dt.bfloat16, kind="ExternalInput")
    corrL = nc.dram_tensor("corrL", [6, 64], dt.bfloat16, kind="ExternalInput")
    corrR = nc.dram_tensor("corrR", [6, OS], dt.bfloat16, kind="ExternalInput")
    sc = nc.dram_tensor("sc", [B, OS], dt.float32, kind="ExternalInput")
    out = nc.dram_tensor("out", [B, OS], dt.float32, kind="ExternalOutput")

    with tile.TileContext(nc) as tc:
        with (
            tc.tile_pool(name="consts", bufs=1) as cpool,
            tc.tile_pool(name="qp", bufs=3) as qpool,
            tc.tile_pool(name="tp", bufs=2) as tpool,
            tc.tile_pool(name="wp", bufs=3) as wpool,
            tc.tile_pool(name="op", bufs=2) as opool,
            tc.tile_pool(name="ps", bufs=1, space="PSUM") as pspool,
        ):
            statA_sb = cpool.tile([128, KT * 64], dt.bfloat16, name="statA_sb")
            statC_sb = cpool.tile([128, KT * 64], dt.bfloat16, name="statC_sb")
            corrL_sb = cpool.tile([6, 64], dt.bfloat16, name="corrL_sb")
            corrR_sb = cpool.tile([6, OS], dt.bfloat16, name="corrR_sb")
            sc_sb = cpool.tile([B, OS], dt.float32, name="sc_sb")
            nc.sync.dma_start(statA_sb, statA[:, :])
            nc.sync.dma_start(statC_sb, statC[:, :])
            nc.sync.dma_start(corrL_sb, corrL[:, :])
            nc.sync.dma_start(corrR_sb, corrR[:, :])
            nc.sync.dma_start(sc_sb, sc[:, :])

            psums = [
                pspool.tile([64, n], dt.float32, name=f"ps{i}")
                for i, (s, n) in enumerate(BLKS)
            ]

            for kt in range(KT):
                qt = qpool.tile([128, HOS], dt.uint16, name="qt", tag="qt")
                nc.sync.dma_start(qt, q[kt * 128 : (kt + 1) * 128, :])
                qb = wpool.tile([128, OS], dt.bfloat16, name="qb", tag="qb")
                hb = wpool.tile([128, OS], dt.bfloat16, name="hb", tag="hb")
                hbu = hb.bitcast(dt.uint16)
                t1 = tpool.tile([128, HOS], dt.uint16, name="t1", tag="t1")
                t2 = tpool.tile([128, HOS], dt.uint16, name="t2", tag="t2")
                # q-cast (exact bf16 of 0..255); interleaved column order
                if kt < N_ACT_CAST:
                    nc.scalar.activation(
                        qb, qt.bitcast(dt.uint8), mybir.ActivationFunctionType.Copy
                    )
                else:
                    nc.gpsimd.tensor_copy(qb, qt.bitcast(dt.uint8))
                # h-stream bit trick: bf16 bits 0x4300|(h<<3) = 128+8h
                nc.vector.tensor_scalar(t1, qt, 1, None, op.logical_shift_right)
                nc.vector.tensor_scalar(
                    hbu[:, 0:HOS], t1, 0x78, 0x4300, op.bitwise_and, op.bitwise_or
                )
                nc.vector.tensor_scalar(
                    t2, t1, 8, 0x78, op.logical_shift_right, op.bitwise_and
                )
                nc.vector.tensor_scalar(
                    hbu[:, HOS:OS], t2, 0x4300, None, op.bitwise_or
                )
                for i, (s, n) in enumerate(BLKS):
                    nc.tensor.matmul(
                        psums[i],
                        statA_sb[:, kt * 64 : kt * 64 + 64],
                        _qcast_ap(qb, s, n),
                        start=(kt == 0),
                        stop=False,
                    )
                    nc.tensor.matmul(
                        psums[i],
                        statC_sb[:, kt * 64 : kt * 64 + 64],
                        hb[:, s : s + n],
                        start=False,
                        stop=False,
                    )

            for i, (s, n) in enumerate(BLKS):
                # rank-1 corrections: -zeros*rowsum and -128*sum(c')
                nc.tensor.matmul(
                    psums[i],
                    corrL_sb,
                    corrR_sb[:, s : s + n],
                    start=False,
                    stop=True,
                )
                t0 = opool.tile([B, n], dt.float32, name="t0", tag=f"t0{i}")
                t = opool.tile([B, n], dt.float32, name="t", tag=f"t{i}")
                o = opool.tile([B, n], dt.float32, name="o", tag=f"o{i}")
                # lo-group psum -> sbuf on ACT (only one psum read allowed per TT)
                nc.scalar.activation(
                    t0, psums[i][32:48, :], mybir.ActivationFunctionType.Copy
                )
                nc.vector.tensor_tensor(t, psums[i][0:16, :], t0, op.add)
                nc.vector.tensor_tensor(o, t, sc_sb[:, s : s + n], op.mult)
                nc.sync.dma_start(out[:, s : s + n], o)

    nc.finalize()
    return nc


def _get_program():
    if "nc" not in _CACHE:
        _CACHE["nc"] = _build_program()
    return _CACHE["nc"]


def _host_prep(inp, quant_weight, scales, zeros):
    """Build per-core input maps (layout/precision prep only, no dequant math)."""
    inp64 = np.asarray(inp, dtype=np.float64)
    a = inp64[:, 0::2].T.copy()  # [HALF, B] even-i activations (pair with l)
    b = inp64[:, 1::2].T.copy()  # [HALF, B] odd-i activations (pair with h)
    # q-stream pairs with a; bit-trick h-stream pairs with c' = (b-16a)/8
    cp = (b - 16.0 * a) / 8.0
    a_hi, a_lo = _split_hi_lo(a)
    c_hi, c_lo = _split_hi_lo(cp)

    statA = np.zeros((128, KT * 64), dtype=BF16)
    statC = np.zeros((128, KT * 64), dtype=BF16)
    for kt in range(KT):
        rows = slice(kt * 128, (kt + 1) * 128)
        statA[:, kt * 64 : kt * 64 + 16] = a_hi[rows]
        statA[:, kt * 64 + 32 : kt * 64 + 48] = a_lo[rows]
        statC[:, kt * 64 : kt * 64 + 16] = c_hi[rows]
        statC[:, kt * 64 + 32 : kt * 64 + 48] = c_lo[rows]

    rowsum = inp64.sum(axis=1)  # [B]
    rs_hi, rs_lo = _split_hi_lo(rowsum)
    s_c = cp.sum(axis=0)  # [B]  sum_j c'[j,b]
    sc_hi, sc_lo = _split_hi_lo(s_c)
    corrL = np.zeros((6, 64), dtype=BF16)
    corrL[0, :16] = rs_hi
    corrL[1, :16] = rs_hi
    corrL[2, :16] = rs_lo
    corrL[3, :16] = rs_lo
    corrL[4, :16] = sc_hi
    corrL[5, :16] = sc_lo

    qw = np.asarray(quant_weight)
    scales = np.asarray(scales, dtype=np.float64).reshape(-1)
    zeros = np.asarray(zeros, dtype=np.float64).reshape(-1)

    in_maps = []
    for cidx in range(NCORES):
        rows = slice(cidx * OS, (cidx + 1) * OS)
        qc = qw[rows].astype(np.uint8).T  # [HALF, OS] natural columns
        # byte-pair columns (m, 688+m) -> uint16 elements
        qc2 = np.empty((HALF, OS), dtype=np.uint8)
        qc2[:, 0::2] = qc[:, :HOS]
        qc2[:, 1::2] = qc[:, HOS:]
        qu16 = np.ascontiguousarray(qc2).view(np.uint16)  # [HALF, HOS]
        z = zeros[rows]
        z_hi, z_lo = _split_hi_lo(z)
        corrR = np.zeros((6, OS), dtype=BF16)
        corrR[0] = -z_hi
        corrR[1] = -z_lo
        corrR[2] = -z_hi
        corrR[3] = -z_lo
        corrR[4] = -128.0
        corrR[5] = -128.0
        sc_c = np.broadcast_to(scales[rows].astype(np.float32), (B, OS)).copy()
        in_maps.append(
            {
                "q": qu16,
                "statA": statA,
                "statC": statC,
                "corrL": corrL,
                "corrR": corrR,
                "sc": sc_c,
            }
        )
    return in_maps


def kernel(inp, quant_weight, scales, zeros):
    from concourse.bass_utils import run_bass_kernel_spmd

    nc = _get_program()
    in_maps = _host_prep(inp, quant_weight, scales, zeros)
    res = run_bass_kernel_spmd(nc, in_maps, core_ids=list(range(NCORES)))
    out = np.concatenate(
        [res.results[c]["out"] for c in range(NCORES)], axis=1
    )
    return np.ascontiguousarray(out.astype(np.float32))



# revision 3
# speedup vs baseline: 1.7833x; 1.7833x over previous
"""4-bit column-block-quantized linear (ColBlockQuantizedLinear) on 8 TRN2 cores.

Math:  out[b,o] = scales[o] * (sum_i inp[b,i]*w[o,i] - zeros[o]*rowsum[b])
where w comes from packed bytes q[o,j] (j = i//2): even i -> low nibble l,
odd i -> high nibble h.

Device-side trick: an e3m4 (float8e3) value with bit pattern 0x60|v equals
8 + v/2 exactly for the FULL nibble range v in 0..15 (fixed exponent 2^3,
4 mantissa bits).  So both nibble streams are produced by cheap DVE bit ops
(no ACT casts, no GPSIMD):
    L = (q & 0x0f0f) | 0x6060          (one dual-ALU tensor_scalar)
    H = ((q >> 4) & 0x0f0f) | 0x6060   (two tensor_scalar ops)
and the PE runs mixed-precision matmuls: bf16 activations (stationary,
hi/lo split for ~fp32 accuracy) x e3m4 nibble streams (moving).

With l = 2L-16, h = 2H-16:
    sum_i inp*w = 2*(A_e . L + A_o . H) - 16*rowsum
    out = (2*scales) * (S - (8 + zeros/2)*rowsum)
The rank-1 (8+zeros/2)*rowsum term is a K=4 bf16 correction matmul
(hi/lo-split factors) accumulated into the same PSUM group.

Host byte layout: per core the packed bytes [2048, 1376] are viewed as
uint16 pairs of ADJACENT columns (2m, 2m+1), so the DVE-produced fp8 bytes
land in natural contiguous column order (no strided matmul APs).

Sharding: column-parallel over out_features (1376 rows/core), inputs
replicated; per-core output [16,1376] gathered on host.
"""

import numpy as np
import ml_dtypes

B = 16
I = 4096
O = 11008
NCORES = 8
OS = O // NCORES          # 1376 out-features per core
HALF = I // 2             # 2048 packed-byte rows (contraction dim per stream)
KT = HALF // 128          # 16 contraction tiles
HOS = OS // 2             # 688 u16 columns per packed tile
BLKS = [(0, 512), (512, 512), (1024, 352)]   # psum-bank o-blocks
SPLIT = 2                 # activation bf16 levels (hi/lo)
M = 64 if SPLIT == 2 else 16   # stationary cols (lo group at partition 32)

BF16 = ml_dtypes.bfloat16

_CACHE = {}


def _split_hi_lo(x64):
    """Split float64 array into (hi, lo) bf16 parts: hi+lo ~= x to ~2^-17."""
    hi = x64.astype(BF16)
    lo = (x64 - hi.astype(np.float64)).astype(BF16)
    return hi, lo


def _build_program():
    import concourse.bacc as bacc
    import concourse.mybir as mybir
    import concourse.tile as tile

    dt = mybir.dt
    op = mybir.AluOpType
    nc = bacc.Bacc("TRN2", target_bir_lowering=False)

    q = nc.dram_tensor("q", [HALF, HOS], dt.uint16, kind="ExternalInput")
    statE = nc.dram_tensor("statE", [128, KT * M], dt.bfloat16, kind="ExternalInput")
    statO = nc.dram_tensor("statO", [128, KT * M], dt.bfloat16, kind="ExternalInput")
    corrL = nc.dram_tensor("corrL", [4, M], dt.bfloat16, kind="ExternalInput")
    corrR = nc.dram_tensor("corrR", [4, OS], dt.bfloat16, kind="ExternalInput")
    sc = nc.dram_tensor("sc", [B, OS], dt.float32, kind="ExternalInput")
    out = nc.dram_tensor("out", [B, OS], dt.float32, kind="ExternalOutput")

    with tile.TileContext(nc) as tc:
        with (
            tc.tile_pool(name="consts", bufs=1) as cpool,
            tc.tile_pool(name="qp", bufs=3) as qpool,
            tc.tile_pool(name="wp", bufs=3) as wpool,
            tc.tile_pool(name="op", bufs=2) as opool,
            tc.tile_pool(name="ps", bufs=1, space="PSUM") as pspool,
        ):
            statE_sb = cpool.tile([128, KT * M], dt.bfloat16, name="statE_sb")
            statO_sb = cpool.tile([128, KT * M], dt.bfloat16, name="statO_sb")
            corrL_sb = cpool.tile([4, M], dt.bfloat16, name="corrL_sb")
            corrR_sb = cpool.tile([4, OS], dt.bfloat16, name="corrR_sb")
            sc_sb = cpool.tile([B, OS], dt.float32, name="sc_sb")
            nc.sync.dma_start(statE_sb, statE[:, :])
            nc.sync.dma_start(statO_sb, statO[:, :])
            nc.sync.dma_start(corrL_sb, corrL[:, :])
            nc.sync.dma_start(corrR_sb, corrR[:, :])
            nc.sync.dma_start(sc_sb, sc[:, :])

            psums = [
                pspool.tile([M, n], dt.float32, name=f"ps{i}")
                for i, (s, n) in enumerate(BLKS)
            ]

            for kt in range(KT):
                qt = qpool.tile([128, HOS], dt.uint16, name="qt", tag="qt")
                nc.sync.dma_start(qt, q[kt * 128 : (kt + 1) * 128, :])
                lv = wpool.tile([128, HOS], dt.uint16, name="lv", tag="lv")
                t1 = wpool.tile([128, HOS], dt.uint16, name="t1", tag="t1")
                hv = wpool.tile([128, HOS], dt.uint16, name="hv", tag="hv")
                # e3m4 bit trick: 0x60|v == 8 + v/2 exactly for v in 0..15
                nc.vector.tensor_scalar(
                    lv, qt, 0x0F0F, 0x6060, op.bitwise_and, op.bitwise_or
                )
                nc.vector.tensor_scalar(t1, qt, 4, None, op.logical_shift_right)
                nc.vector.tensor_scalar(
                    hv, t1, 0x0F0F, 0x6060, op.bitwise_and, op.bitwise_or
                )
                lv8 = lv.bitcast(dt.float8e3)
                hv8 = hv.bitcast(dt.float8e3)
                ecols = statE_sb[:, kt * M : (kt + 1) * M]
                ocols = statO_sb[:, kt * M : (kt + 1) * M]
                for i, (s, n) in enumerate(BLKS):
                    nc.tensor.matmul(
                        psums[i], ecols, lv8[:, s : s + n],
                        start=(kt == 0), stop=False,
                    )
                    nc.tensor.matmul(
                        psums[i], ocols, hv8[:, s : s + n],
                        start=False, stop=False,
                    )

            for i, (s, n) in enumerate(BLKS):
                # rank-1 correction: -(8 + zeros/2) * rowsum
                nc.tensor.matmul(
                    psums[i], corrL_sb, corrR_sb[:, s : s + n],
                    start=False, stop=True,
                )
                t0 = opool.tile([B, n], dt.float32, name="t0", tag=f"t0{i}")
                t = opool.tile([B, n], dt.float32, name="t", tag=f"t{i}")
                o = opool.tile([B, n], dt.float32, name="o", tag=f"o{i}")
                # lo-group psum -> sbuf on ACT (only one psum read allowed per TT)
                nc.scalar.activation(
                    t0, psums[i][32:48, :], mybir.ActivationFunctionType.Copy
                )
                nc.vector.tensor_tensor(t, psums[i][0:16, :], t0, op.add)
                nc.vector.tensor_tensor(o, t, sc_sb[:, s : s + n], op.mult)
                nc.sync.dma_start(out[:, s : s + n], o)

    nc.finalize()
    return nc


def _get_program():
    if "nc" not in _CACHE:
        _CACHE["nc"] = _build_program()
    return _CACHE["nc"]


def _host_prep(inp, quant_weight, scales, zeros):
    """Build per-core input maps (layout/precision prep only, no dequant math)."""
    inp64 = np.asarray(inp, dtype=np.float64)
    a_e = inp64[:, 0::2].T.copy()  # [HALF, B] even-i activations (pair with L)
    a_o = inp64[:, 1::2].T.copy()  # [HALF, B] odd-i activations (pair with H)
    e_hi, e_lo = _split_hi_lo(a_e)
    o_hi, o_lo = _split_hi_lo(a_o)

    statE = np.zeros((128, KT * M), dtype=BF16)
    statO = np.zeros((128, KT * M), dtype=BF16)
    for kt in range(KT):
        rows = slice(kt * 128, (kt + 1) * 128)
        statE[:, kt * M : kt * M + 16] = e_hi[rows]
        statO[:, kt * M : kt * M + 16] = o_hi[rows]
        if SPLIT == 2:
            statE[:, kt * M + 32 : kt * M + 48] = e_lo[rows]
            statO[:, kt * M + 32 : kt * M + 48] = o_lo[rows]

    rowsum = inp64.sum(axis=1)  # [B]
    rs_hi, rs_lo = _split_hi_lo(rowsum)
    corrL = np.zeros((4, M), dtype=BF16)
    corrL[0, :16] = rs_hi
    corrL[1, :16] = rs_lo
    corrL[2, :16] = rs_hi
    corrL[3, :16] = rs_lo

    qw = np.asarray(quant_weight)
    scales = np.asarray(scales, dtype=np.float64).reshape(-1)
    zeros = np.asarray(zeros, dtype=np.float64).reshape(-1)

    in_maps = []
    for cidx in range(NCORES):
        rows = slice(cidx * OS, (cidx + 1) * OS)
        qc = qw[rows].astype(np.uint8).T  # [HALF, OS] natural columns
        qu16 = np.ascontiguousarray(qc).view(np.uint16)  # pairs (2m, 2m+1)
        z2 = 8.0 + zeros[rows] / 2.0
        z2_hi, z2_lo = _split_hi_lo(z2)
        corrR = np.zeros((4, OS), dtype=BF16)
        corrR[0] = -z2_hi
        corrR[1] = -z2_hi
        corrR[2] = -z2_lo
        corrR[3] = -z2_lo
        sc_c = np.broadcast_to(
            2.0 * scales[rows].astype(np.float32), (B, OS)
        ).copy()
        in_maps.append(
            {
                "q": qu16,
                "statE": statE,
                "statO": statO,
                "corrL": corrL,
                "corrR": corrR,
                "sc": sc_c,
            }
        )
    return in_maps


def kernel(inp, quant_weight, scales, zeros):
    from concourse.bass_utils import run_bass_kernel_spmd

    nc = _get_program()
    in_maps = _host_prep(inp, quant_weight, scales, zeros)
    res = run_bass_kernel_spmd(nc, in_maps, core_ids=list(range(NCORES)))
    out = np.concatenate(
        [res.results[c]["out"] for c in range(NCORES)], axis=1
    )
    return np.ascontiguousarray(out.astype(np.float32))


# revision 4
# speedup vs baseline: 1.7945x; 1.0063x over previous
"""4-bit column-block-quantized linear (ColBlockQuantizedLinear) on 8 TRN2 cores.

Math:  out[b,o] = scales[o] * (sum_i inp[b,i]*w[o,i] - zeros[o]*rowsum[b])
where w comes from packed bytes q[o,j] (j = i//2): even i -> low nibble l,
odd i -> high nibble h.

Device-side trick: an e3m4 (float8e3) value with bit pattern 0x60|v equals
8 + v/2 exactly for the FULL nibble range v in 0..15 (fixed exponent 2^3,
4 mantissa bits).  So both nibble streams are produced by cheap DVE bit ops
(no ACT casts, no GPSIMD):
    L = (q & 0x0f0f) | 0x6060          (one dual-ALU tensor_scalar)
    H = ((q >> 4) & 0x0f0f) | 0x6060   (two tensor_scalar ops)
and the PE runs mixed-precision matmuls: bf16 activations (stationary,
hi/lo split for ~fp32 accuracy) x e3m4 nibble streams (moving).

With l = 2L-16, h = 2H-16:
    sum_i inp*w = 2*(A_e . L + A_o . H) - 16*rowsum
    out = (2*scales) * (S - (8 + zeros/2)*rowsum)
The rank-1 (8+zeros/2)*rowsum term is a K=4 bf16 correction matmul
(hi/lo-split factors) accumulated into the same PSUM group.

Host byte layout: per core the packed bytes [2048, 1376] are viewed as
uint16 pairs of ADJACENT columns (2m, 2m+1), so the DVE-produced fp8 bytes
land in natural contiguous column order (no strided matmul APs).

Sharding: column-parallel over out_features (1376 rows/core), inputs
replicated; per-core output [16,1376] gathered on host.
"""

import numpy as np
import ml_dtypes

B = 16
I = 4096
O = 11008
NCORES = 8
OS = O // NCORES          # 1376 out-features per core
HALF = I // 2             # 2048 packed-byte rows (contraction dim per stream)
KT = HALF // 128          # 16 contraction tiles
HOS = OS // 2             # 688 u16 columns per packed tile
BLKS = [(0, 512), (512, 512), (1024, 352)]   # psum-bank o-blocks
SPLIT = 2                 # activation bf16 levels (hi/lo)
M = 64 if SPLIT == 2 else 16   # stationary cols (lo group at partition 32)

BF16 = ml_dtypes.bfloat16

_CACHE = {}


def _split_hi_lo(x64):
    """Split float64 array into (hi, lo) bf16 parts: hi+lo ~= x to ~2^-17."""
    hi = x64.astype(BF16)
    lo = (x64 - hi.astype(np.float64)).astype(BF16)
    return hi, lo


def _build_program():
    import concourse.bacc as bacc
    import concourse.mybir as mybir
    import concourse.tile as tile

    dt = mybir.dt
    op = mybir.AluOpType
    nc = bacc.Bacc("TRN2", target_bir_lowering=False)

    q = nc.dram_tensor("q", [HALF, HOS], dt.uint16, kind="ExternalInput")
    statE = nc.dram_tensor("statE", [128, KT * M], dt.bfloat16, kind="ExternalInput")
    statO = nc.dram_tensor("statO", [128, KT * M], dt.bfloat16, kind="ExternalInput")
    corrL = nc.dram_tensor("corrL", [4, M], dt.bfloat16, kind="ExternalInput")
    corrR = nc.dram_tensor("corrR", [4, OS], dt.bfloat16, kind="ExternalInput")
    sc = nc.dram_tensor("sc", [B, OS], dt.float32, kind="ExternalInput")
    out = nc.dram_tensor("out", [B, OS], dt.float32, kind="ExternalOutput")

    with tile.TileContext(nc) as tc:
        with (
            tc.tile_pool(name="consts", bufs=1) as cpool,
            tc.tile_pool(name="qp", bufs=3) as qpool,
            tc.tile_pool(name="wp", bufs=3) as wpool,
            tc.tile_pool(name="op", bufs=2) as opool,
            tc.tile_pool(name="ps", bufs=1, space="PSUM") as pspool,
        ):
            statE_sb = cpool.tile([128, KT * M], dt.bfloat16, name="statE_sb")
            statO_sb = cpool.tile([128, KT * M], dt.bfloat16, name="statO_sb")
            corrL_sb = cpool.tile([4, M], dt.bfloat16, name="corrL_sb")
            corrR_sb = cpool.tile([4, OS], dt.bfloat16, name="corrR_sb")
            sc_sb = cpool.tile([B, OS], dt.float32, name="sc_sb")
            nc.gpsimd.dma_start(statE_sb, statE[:, :])
            nc.gpsimd.dma_start(statO_sb, statO[:, :])
            nc.gpsimd.dma_start(corrL_sb, corrL[:, :])
            nc.gpsimd.dma_start(corrR_sb, corrR[:, :])
            nc.gpsimd.dma_start(sc_sb, sc[:, :])

            psums = [
                pspool.tile([M, n], dt.float32, name=f"ps{i}")
                for i, (s, n) in enumerate(BLKS)
            ]

            for kt in range(KT):
                qt = qpool.tile([128, HOS], dt.uint16, name="qt", tag="qt")
                nc.sync.dma_start(qt, q[kt * 128 : (kt + 1) * 128, :])
                lv = wpool.tile([128, HOS], dt.uint16, name="lv", tag="lv")
                t1 = wpool.tile([128, HOS], dt.uint16, name="t1", tag="t1")
                hv = wpool.tile([128, HOS], dt.uint16, name="hv", tag="hv")
                # e3m4 bit trick: 0x60|v == 8 + v/2 exactly for v in 0..15
                nc.vector.tensor_scalar(
                    lv, qt, 0x0F0F, 0x6060, op.bitwise_and, op.bitwise_or
                )
                nc.vector.tensor_scalar(t1, qt, 4, None, op.logical_shift_right)
                nc.vector.tensor_scalar(
                    hv, t1, 0x0F0F, 0x6060, op.bitwise_and, op.bitwise_or
                )
                lv8 = lv.bitcast(dt.float8e3)
                hv8 = hv.bitcast(dt.float8e3)
                ecols = statE_sb[:, kt * M : (kt + 1) * M]
                ocols = statO_sb[:, kt * M : (kt + 1) * M]
                for i, (s, n) in enumerate(BLKS):
                    nc.tensor.matmul(
                        psums[i], ecols, lv8[:, s : s + n],
                        start=(kt == 0), stop=False,
                    )
                    nc.tensor.matmul(
                        psums[i], ocols, hv8[:, s : s + n],
                        start=False, stop=False,
                    )

            for i, (s, n) in enumerate(BLKS):
                # rank-1 correction: -(8 + zeros/2) * rowsum
                nc.tensor.matmul(
                    psums[i], corrL_sb, corrR_sb[:, s : s + n],
                    start=False, stop=True,
                )
                t0 = opool.tile([B, n], dt.float32, name="t0", tag=f"t0{i}")
                t = opool.tile([B, n], dt.float32, name="t", tag=f"t{i}")
                o = opool.tile([B, n], dt.float32, name="o", tag=f"o{i}")
                # lo-group psum -> sbuf on ACT (only one psum read allowed per TT)
                nc.scalar.activation(
                    t0, psums[i][32:48, :], mybir.ActivationFunctionType.Copy
                )
                nc.vector.tensor_tensor(t, psums[i][0:16, :], t0, op.add)
                nc.vector.tensor_tensor(o, t, sc_sb[:, s : s + n], op.mult)
                nc.gpsimd.dma_start(out[:, s : s + n], o)

    nc.finalize()
    return nc


def _get_program():
    if "nc" not in _CACHE:
        _CACHE["nc"] = _build_program()
    return _CACHE["nc"]


def _host_prep(inp, quant_weight, scales, zeros):
    """Build per-core input maps (layout/precision prep only, no dequant math)."""
    inp64 = np.asarray(inp, dtype=np.float64)
    a_e = inp64[:, 0::2].T.copy()  # [HALF, B] even-i activations (pair with L)
    a_o = inp64[:, 1::2].T.copy()  # [HALF, B] odd-i activations (pair with H)
    e_hi, e_lo = _split_hi_lo(a_e)
    o_hi, o_lo = _split_hi_lo(a_o)

    statE = np.zeros((128, KT * M), dtype=BF16)
    statO = np.zeros((128, KT * M), dtype=BF16)
    for kt in range(KT):
        rows = slice(kt * 128, (kt + 1) * 128)
        statE[:, kt * M : kt * M + 16] = e_hi[rows]
        statO[:, kt * M : kt * M + 16] = o_hi[rows]
        if SPLIT == 2:
            statE[:, kt * M + 32 : kt * M + 48] = e_lo[rows]
            statO[:, kt * M + 32 : kt * M + 48] = o_lo[rows]

    rowsum = inp64.sum(axis=1)  # [B]
    rs_hi, rs_lo = _split_hi_lo(rowsum)
    corrL = np.zeros((4, M), dtype=BF16)
    corrL[0, :16] = rs_hi
    corrL[1, :16] = rs_lo
    corrL[2, :16] = rs_hi
    corrL[3, :16] = rs_lo

    qw = np.asarray(quant_weight)
    scales = np.asarray(scales, dtype=np.float64).reshape(-1)
    zeros = np.asarray(zeros, dtype=np.float64).reshape(-1)

    in_maps = []
    for cidx in range(NCORES):
        rows = slice(cidx * OS, (cidx + 1) * OS)
        qc = qw[rows].astype(np.uint8).T  # [HALF, OS] natural columns
        qu16 = np.ascontiguousarray(qc).view(np.uint16)  # pairs (2m, 2m+1)
        z2 = 8.0 + zeros[rows] / 2.0
        z2_hi, z2_lo = _split_hi_lo(z2)
        corrR = np.zeros((4, OS), dtype=BF16)
        corrR[0] = -z2_hi
        corrR[1] = -z2_hi
        corrR[2] = -z2_lo
        corrR[3] = -z2_lo
        sc_c = np.broadcast_to(
            2.0 * scales[rows].astype(np.float32), (B, OS)
        ).copy()
        in_maps.append(
            {
                "q": qu16,
                "statE": statE,
                "statO": statO,
                "corrL": corrL,
                "corrR": corrR,
                "sc": sc_c,
            }
        )
    return in_maps


def kernel(inp, quant_weight, scales, zeros):
    from concourse.bass_utils import run_bass_kernel_spmd

    nc = _get_program()
    in_maps = _host_prep(inp, quant_weight, scales, zeros)
    res = run_bass_kernel_spmd(nc, in_maps, core_ids=list(range(NCORES)))
    out = np.concatenate(
        [res.results[c]["out"] for c in range(NCORES)], axis=1
    )
    return np.ascontiguousarray(out.astype(np.float32))
